# revision 32
# baseline (speedup 1.0000x reference)
"""Trainium2 Bass kernel for nn_CLFormer (3-block linear-attention transformer).

Sharding: pure data parallel — batch 32 split as 4 per NeuronCore across 8
cores; all parameters replicated; outputs concatenated.

Per-core layout: 4 batches x 32 channels on the 128 SBUF partitions
("channel-major" [128=4bx32c, L]). Per block:
  phase 1 (token-major): h slices are transposed SBUF->SBUF via the DMA
    xbar; E = exp(h) on ScalarE; the gram G = E^T h and the k-softmax
    denominator are fused into one PE matmul per chunk by extending the
    moving operand with a ones column (129 cols); q = E / rowsum(E) with
    the reciprocal on the DVE (reciprocal_approx_fast) and the multiply on
    GPSIMD; q is transposed back channel-major via the DMA xbar.
  phase 2 (channel-major): z1 = blockdiag(M1)^T q, gelu, z2 =
    blockdiag(W2)^T a1, gelu — one matmul per 512 tokens using
    block-diagonal stationaries (M1 = diag(1/ksum) G_masked W1 folds the
    attention output projection into FC1).
Activation LUT sets: Sqrt once at start, then per block Exp (phase 1) and
Gelu (phase 2); a fence tile makes each block's exps depend on the previous
block's last gelu so the scalar queue never interleaves the two sets.
"""
import sys
import numpy as np

for _p in ("/opt/trn_rl_repo", "/root/.axon_site/_ro/trn_rl_repo"):
    if _p not in sys.path:
        sys.path.append(_p)

from contextlib import ExitStack

import concourse.bass as bass
import concourse.mybir as mybir
import bass_rust
from concourse import tile
from concourse.masks import make_identity
from concourse.bass_utils import run_bass_kernel_spmd

F32 = mybir.dt.float32
BF16 = mybir.dt.bfloat16
AF = mybir.ActivationFunctionType
MUL = mybir.AluOpType.mult
ADD = mybir.AluOpType.add

P = 128
B_LOC = 4            # batches per core
C = 32               # channels
L = 16384            # sequence length
NB = 3               # transformer blocks
DOUT = 10
HEADS = 4
DH = 8
BN_EPS = 1e-5

SLC = 2048           # tokens per phase-1 slice
NSL = L // SLC       # 8 slices
NCH = SLC // 128     # 16 chunks per slice
ZW = 1024            # tokens per phase-2 chunk
NZ = L // ZW         # 16 chunks
EXT = 129            # used chunk width (128 h cols + 1 ones col)
PITCH = 144          # chunk pitch of the extended tile (xbar wants 16-elem mult)


# ---------------------------------------------------------------- waitfix --
_WF_SKIP = {"InstEventSemaphore"}
_wf_ctr = [0]


def _fix_sync_waits(nc):
    """Hoist excess sync waits onto InstEventSemaphore (this walrus build
    accepts only 1 wait per instruction). The event-sem executes on the same
    engine stream immediately before, preserving semantics."""
    for fn in nc.m.functions:
        new_blocks = []
        for blk in fn.blocks:
            out = []
            for ins in blk.instructions:
                tname = type(ins).__name__
                si = ins.sync_info
                if si is None or tname in _WF_SKIP:
                    out.append(ins)
                    continue
                waits = list(si.on_wait)
                if len(waits) <= 1:
                    out.append(ins)
                    continue
                keep = waits[-1:]
                excess = waits[:-1]
                for i in range(0, len(excess), 2):
                    chunk = excess[i:i + 2]
                    _wf_ctr[0] += 1
                    ev = mybir.InstEventSemaphore(
                        name=f"wfix{_wf_ctr[0]}", ins=[], outs=[])
                    ev.engine = ins.engine
                    ev.sync_info = mybir.SyncInfo(on_wait=chunk, on_update=[])
                    out.append(ev)
                ins.sync_info = mybir.SyncInfo(
                    on_wait=keep, on_update=list(si.on_update))
                out.append(ins)
            nb = bass_rust.BasicBlock(name=blk.name, instructions=out)
            new_blocks.append(nb)
        fn.blocks = new_blocks


# ---------------------------------------------------------------- program --
def _load_rep(nc, pool, src_ap, cols, tag, eng=None):
    """DRAM [32, cols] view -> SBUF [128, cols] f32, replicated across the 4
    batch partition strips (one DMA per strip)."""
    stage = pool.tile([P, cols], F32, tag=tag, name=tag)
    eng = eng if eng is not None else nc.sync
    for b in range(B_LOC):
        eng.dma_start(stage[C * b:C * (b + 1), :], src_ap)
    return stage


DEBUG = False


def build_program(reps=1):
    nc = bass.Bass()

    x_d = nc.declare_dram_parameter("x", [B_LOC, C, L], F32, isOutput=False)
    fcW1_d = nc.declare_dram_parameter("fcW1", [NB, C, C], F32, isOutput=False)
    fcb1_d = nc.declare_dram_parameter("fcb1", [NB, C], F32, isOutput=False)
    fcW2_d = nc.declare_dram_parameter("fcW2", [NB, C, C], F32, isOutput=False)
    fcb2_d = nc.declare_dram_parameter("fcb2", [NB, C], F32, isOutput=False)
    Wh_d = nc.declare_dram_parameter("Wh", [C, C], F32, isOutput=False)
    bh_d = nc.declare_dram_parameter("bh", [C], F32, isOutput=False)
    bng_d = nc.declare_dram_parameter("bn_gamma", [C], F32, isOutput=False)
    bnb_d = nc.declare_dram_parameter("bn_beta", [C], F32, isOutput=False)
    bnm_d = nc.declare_dram_parameter("bn_mean", [C], F32, isOutput=False)
    bnv_d = nc.declare_dram_parameter("bn_var", [C], F32, isOutput=False)
    Wf_d = nc.declare_dram_parameter("Wf", [C, DOUT], F32, isOutput=False)
    bf_d = nc.declare_dram_parameter("bf", [DOUT], F32, isOutput=False)
    out_d = nc.declare_dram_parameter("out", [B_LOC, DOUT], F32, isOutput=True)
    if DEBUG:
        dbg_G = [nc.declare_dram_parameter(f"dbg_G{i}", [P, EXT], F32,
                                           isOutput=True) for i in range(NB)]
        dbg_M1 = [nc.declare_dram_parameter(f"dbg_M1_{i}", [P, C], BF16,
                                            isOutput=True) for i in range(NB)]
        dbg_h = [nc.declare_dram_parameter(f"dbg_h{i}", [P, ZW], BF16,
                                           isOutput=True) for i in range(NB)]
        dbg_q = [nc.declare_dram_parameter(f"dbg_q{i}", [P, ZW], BF16,
                                           isOutput=True) for i in range(NB)]
        dbg_pool = nc.declare_dram_parameter("dbg_pool", [P, NZ], F32,
                                             isOutput=True)
        dbg_W1 = nc.declare_dram_parameter("dbg_W1", [P, NB * C], BF16,
                                           isOutput=True)
        dbg_ksC = [nc.declare_dram_parameter(f"dbg_ksC{i}", [P, 1], F32,
                                             isOutput=True) for i in range(NB)]
        dbg_S1 = [nc.declare_dram_parameter(f"dbg_S1_{i}", [P, P], BF16,
                                            isOutput=True) for i in range(NB)]

    with ExitStack() as ctx:
        tc = ctx.enter_context(tile.TileContext(nc))
        cst = ctx.enter_context(tc.tile_pool(name="cst", bufs=1))
        big = ctx.enter_context(tc.tile_pool(name="big", bufs=1))
        xst = ctx.enter_context(tc.tile_pool(name="xst", bufs=2))
        hexp = ctx.enter_context(tc.tile_pool(name="hexp", bufs=3))
        etp = ctx.enter_context(tc.tile_pool(name="etp", bufs=3))
        qtp = ctx.enter_context(tc.tile_pool(name="qtp", bufs=3))
        sqp = ctx.enter_context(tc.tile_pool(name="sqp", bufs=2))
        a1p = ctx.enter_context(tc.tile_pool(name="a1p", bufs=3))
        smal = ctx.enter_context(tc.tile_pool(name="smal", bufs=2))
        gps = ctx.enter_context(tc.tile_pool(name="gps", bufs=1, space="PSUM"))
        zp1 = ctx.enter_context(tc.tile_pool(name="zp1", bufs=2, space="PSUM"))
        zp2 = ctx.enter_context(tc.tile_pool(name="zp2", bufs=1, space="PSUM"))
        tps = ctx.enter_context(tc.tile_pool(name="tps", bufs=1, space="PSUM"))

        # ---- BN eval folding first: its Sqrt claims the scalar LUT before
        # the exp/gelu sets start cycling.
        # bn vectors ride the scalar HWDGE queue (idle at prologue)
        bh_r = _load_rep(nc, cst, bh_d[:].unsqueeze(-1), 1, "bh", nc.scalar)
        bng_r = _load_rep(nc, cst, bng_d[:].unsqueeze(-1), 1, "bng", nc.scalar)
        bnb_r = _load_rep(nc, cst, bnb_d[:].unsqueeze(-1), 1, "bnb", nc.scalar)
        bnm_r = _load_rep(nc, cst, bnm_d[:].unsqueeze(-1), 1, "bnm", nc.scalar)
        bnv_r = _load_rep(nc, cst, bnv_d[:].unsqueeze(-1), 1, "bnv", nc.scalar)
        eps_t = cst.tile([P, 1], F32)
        nc.vector.memset(eps_t[:], BN_EPS)
        sq_t = cst.tile([P, 1], F32)
        nc.scalar.activation(sq_t[:], bnv_r[:], AF.Sqrt, bias=eps_t[:])
        rs_t = cst.tile([P, 1], F32)
        nc.vector.reciprocal(rs_t[:], sq_t[:])
        svec = cst.tile([P, 1], F32)
        nc.vector.tensor_tensor(svec[:], rs_t[:], bng_r[:], op=MUL)
        svecL = cst.tile([P, 1], F32)
        nc.vector.tensor_scalar_mul(svecL[:], svec[:], 1.0 / L)
        t0 = cst.tile([P, 1], F32)
        nc.vector.tensor_tensor(t0[:], bh_r[:], bnm_r[:],
                                op=mybir.AluOpType.subtract)
        t1 = cst.tile([P, 1], F32)
        nc.vector.tensor_tensor(t1[:], t0[:], svec[:], op=MUL)
        tvec = cst.tile([P, 1], F32)
        nc.vector.tensor_tensor(tvec[:], t1[:], bnb_r[:], op=ADD)

        # ---- constants -------------------------------------------------
        ident = cst.tile([P, P], BF16)
        make_identity(nc, ident[:])
        headmask = cst.tile([P, P], BF16)
        nc.vector.memset(headmask[:], 1.0)
        hm_v = headmask[:].rearrange("p (g i) -> p g i", i=DH)
        nc.gpsimd.affine_select(
            out=hm_v, in_=hm_v, pattern=[[-DH, P // DH], [0, DH]],
            compare_op=mybir.AluOpType.is_ge, fill=0.0,
            base=0, channel_multiplier=1)
        nc.gpsimd.affine_select(
            out=hm_v, in_=hm_v, pattern=[[DH, P // DH], [0, DH]],
            compare_op=mybir.AluOpType.is_ge, fill=0.0,
            base=DH - 1, channel_multiplier=-1)

        # ---- params: per-strip DMAs (one per strip covering all blocks) --
        def _load_w(src_d, tag):
            stage = cst.tile([P, NB * C], F32, tag=f"{tag}st", name=f"{tag}st")
            dst = stage[:].rearrange("(b c) (n k) -> b c n k", b=B_LOC, n=NB)
            for b in range(B_LOC):
                nc.sync.dma_start(
                    dst[b], src_d[:].rearrange("n c k -> c n k"))
            rep = cst.tile([P, NB * C], BF16, tag=f"{tag}bf", name=f"{tag}bf")
            nc.vector.tensor_copy(rep[:], stage[:])
            return rep

        W1rep = _load_w(fcW1_d, "w1")
        W2rep = _load_w(fcW2_d, "w2")
        # block-diagonal W2 stationaries
        W2diag = []
        for i in range(NB):
            wd = cst.tile([P, P], BF16, tag=f"w2d{i}", name=f"w2d{i}")
            nc.vector.memset(wd[:], 0.0)
            for b in range(B_LOC):
                sl = slice(C * b, C * (b + 1))
                nc.vector.tensor_copy(
                    wd[sl, C * b:C * (b + 1)], W2rep[sl, C * i:C * (i + 1)])
            W2diag.append(wd)
        def _load_b(src_d, tag):
            stage = cst.tile([P, NB], F32, tag=tag, name=tag)
            for b in range(B_LOC):
                nc.sync.dma_start(
                    stage[C * b:C * (b + 1), :],
                    src_d[:].rearrange("n c -> c n"))
            return stage

        b1rep = _load_b(fcb1_d, "b1")
        b2rep = _load_b(fcb2_d, "b2")
        zero_t = cst.tile([P, 1], F32)
        nc.vector.memset(zero_t[:], 0.0)
        ones_t = cst.tile([P, 1], F32)
        nc.vector.memset(ones_t[:], 1.0)

        # (repetition loop for benchmarking only; reps=1 in production)
        for _rep in range(reps):
            # two persistent channel-major h buffers, ping-ponged per block
            h_tiles = [
                big.tile([P, L], BF16, tag=f"h{i}_{_rep}", name=f"h{i}_{_rep}")
                for i in range(2)
            ]
            q_cm = big.tile([P, L], BF16, tag=f"qcm_{_rep}", name=f"qcm_{_rep}")
            pooled_parts = cst.tile([P, NZ], F32, tag=f"pool_{_rep}",
                                    name=f"pool_{_rep}")

            # ---- input: load x slices, cast to bf16 on the DVE ---------
            # high priority so the x pipeline beats the param-load DMA
            # triggers queued ahead of it on the sync queue
            # x loads ride the scalar HWDGE queue so they don't contend with
            # transposes/params on the sync queue
            x_cm = x_d[:].rearrange("b c l -> (b c) l")
            with tc.high_priority():
                for s in range(NSL):
                    xs = xst.tile([P, SLC], F32, tag="xs", bufs=3)
                    nc.scalar.dma_start(xs[:], x_cm[:, SLC * s:SLC * (s + 1)])
                    nc.vector.tensor_copy(
                        h_tiles[0][:, SLC * s:SLC * (s + 1)], xs[:])

            def _new_hx(src_tile, s):
                hx = hexp.tile([P, NCH, PITCH], BF16, tag="hex", bufs=8,
                               name="hx")
                nc.vector.memset(hx[:, :, P:EXT], 1.0)
                nc.sync.dma_start_transpose(
                    out=hx[:, :, 0:P],
                    in_=src_tile[:, SLC * s:SLC * (s + 1)],
                )
                return hx

            fences = [zero_t, zero_t]
            hx_list = None
            for blk in range(NB):
                h_src = h_tiles[blk % 2]
                h_dst = h_tiles[(blk + 1) % 2]
                G_ps = gps.tile([P, EXT], F32, tag="G")

                # ===================== phase 1 (token-major) ============
                if blk == 0:
                    hx_list = [_new_hx(h_src, s) for s in range(NSL)]

                def _qT(s, qt):
                    # transpose q back to channel-major on the PE
                    # (out = qt_chunk^T @ I), staged through one PSUM bank
                    for g in range(NCH // 4):
                        qp = tps.tile([P, 512], F32, tag="tiny", name="qp")
                        for k in range(4):
                            cc = 4 * g + k
                            nc.tensor.matmul(
                                qp[:, P * k:P * (k + 1)],
                                qt[:, P * cc:P * (cc + 1)], ident[:],
                            )
                        nc.vector.tensor_copy(
                            q_cm[:, SLC * s + 512 * g:SLC * s + 512 * (g + 1)],
                            qp[:],
                        )

                qt_prev = None
                for s in range(NSL):
                    hx = hx_list[s]
                    et = etp.tile([P, SLC], BF16, tag="et")
                    nc.scalar.activation(
                        et[:].rearrange("p (c l) -> p c l", l=P),
                        hx[:, :, 0:P], AF.Exp,
                        bias=fences[0] if s < NSL // 2 else fences[1])
                    # fused gram + ksum: col 128 of the moving operand is 1
                    for c in range(NCH):
                        nc.tensor.matmul(
                            G_ps[:],
                            et[:, P * c:P * (c + 1)],
                            hx[:, c, 0:EXT],
                            start=(s == 0 and c == 0),
                            stop=(s == NSL - 1 and c == NCH - 1),
                        )
                    # previous slice's q transpose after this slice's gram so
                    # the in-order PE queue never stalls waiting on qmul
                    if qt_prev is not None:
                        _qT(s - 1, qt_prev)
                    # q = E / rowsum_d(E)
                    sq = sqp.tile([P, SLC // DH], F32, tag="sq")
                    nc.vector.reduce_sum(
                        sq[:],
                        et[:].rearrange("p (g d) -> p g d", d=DH),
                        axis=mybir.AxisListType.X,
                    )
                    rq = sqp.tile([P, SLC // DH], F32, tag="rq")
                    nc.vector.reciprocal(rq[:], sq[:])
                    qt = qtp.tile([P, SLC], BF16, tag="qt")
                    et_v = et[:].rearrange("p (g d) -> p g d", d=DH)
                    qt_v = qt[:].rearrange("p (g d) -> p g d", d=DH)
                    rq_v = rq[:].unsqueeze(-1)
                    nc.gpsimd.tensor_tensor(
                        qt_v[:], et_v[:],
                        rq_v[:].broadcast_to([P, SLC // DH, DH]),
                        op=MUL,
                    )
                    qt_prev = qt
                _qT(NSL - 1, qt_prev)

                # ===================== M1 build =========================
                ksC = smal.tile([P, 1], F32, tag="ksC")
                nc.vector.reciprocal(ksC[:], G_ps[:, P:EXT])
                G_sb = smal.tile([P, P], BF16, tag="Gsb")
                nc.vector.tensor_tensor(G_sb[:], G_ps[:, 0:P], headmask[:],
                                        op=MUL)
                GT_ps = tps.tile([P, P], F32, tag="tiny")
                nc.tensor.matmul(GT_ps[:], G_sb[:], ident[:])
                GT_sb = smal.tile([P, P], BF16, tag="gtsb")
                nc.vector.tensor_copy(GT_sb[:], GT_ps[:])
                M1u_ps = tps.tile([P, C], F32, tag="tiny")
                for b in range(B_LOC):
                    sl = slice(C * b, C * (b + 1))
                    nc.tensor.matmul(
                        M1u_ps[sl, 0:C], GT_sb[sl, C * b:C * (b + 1)],
                        W1rep[sl, C * blk:C * (blk + 1)],
                        tile_position=(C * b, C * b),
                    )
                M1 = smal.tile([P, C], BF16, tag="m1")
                nc.vector.tensor_scalar_mul(M1[:], M1u_ps[:], ksC[:])
                if DEBUG and _rep == 0:
                    Gf = smal.tile([P, EXT], F32, tag="dbgG")
                    nc.vector.tensor_copy(Gf[:], G_ps[:])
                    nc.sync.dma_start(dbg_G[blk][:], Gf[:])
                    nc.sync.dma_start(dbg_M1[blk][:], M1[:])
                    nc.sync.dma_start(dbg_q[blk][:], q_cm[:, 0:ZW])
                    nc.sync.dma_start(dbg_ksC[blk][:], ksC[:])
                    if blk == 0:
                        nc.sync.dma_start(dbg_W1[:], W1rep[:])
                S1 = smal.tile([P, P], BF16, tag="s1")
                nc.vector.memset(S1[:], 0.0)
                for b in range(B_LOC):
                    sl = slice(C * b, C * (b + 1))
                    nc.vector.tensor_copy(S1[sl, C * b:C * (b + 1)], M1[sl, :])
                if DEBUG and _rep == 0:
                    nc.sync.dma_start(dbg_S1[blk][:], S1[:])

                # ===================== phase 2 (channel-major) ==========
                last = blk == NB - 1
                hx_next = []
                for t in range(NZ):
                    z1 = zp1.tile([P, ZW], F32, tag="z1")
                    for hh in range(ZW // 512):
                        nc.tensor.matmul(
                            z1[:, 512 * hh:512 * (hh + 1)], S1[:],
                            q_cm[:, ZW * t + 512 * hh:ZW * t + 512 * (hh + 1)],
                        )
                    a1 = a1p.tile([P, ZW], BF16, tag="a1")
                    nc.scalar.activation(a1[:], z1[:], AF.Gelu,
                                         bias=b1rep[:, blk:blk + 1])
                    z2 = zp2.tile([P, ZW], F32, tag="z2")
                    for hh in range(ZW // 512):
                        nc.tensor.matmul(
                            z2[:, 512 * hh:512 * (hh + 1)], W2diag[blk][:],
                            a1[:, 512 * hh:512 * (hh + 1)],
                        )
                    if last:
                        nc.scalar.activation(
                            h_dst[:, ZW * t:ZW * (t + 1)], z2[:],
                            AF.Gelu, bias=b2rep[:, blk:blk + 1],
                            accum_out=pooled_parts[:, t:t + 1],
                        )
                    else:
                        nc.scalar.activation(
                            h_dst[:, ZW * t:ZW * (t + 1)], z2[:],
                            AF.Gelu, bias=b2rep[:, blk:blk + 1],
                        )
                        # next block's token-major transpose for slice t//2
                        # fires as soon as its two chunks are written
                        if t % 2 == 1:
                            hx_next.append(_new_hx(h_dst, t // 2))
                        # mid fence: first half of next block's exps may
                        # start once chunks 0..7 are written
                        if t == NZ // 2 - 1:
                            fmid = smal.tile([P, 1], F32, tag="fmid")
                            nc.gpsimd.tensor_tensor(
                                fmid[:], h_dst[:, ZW * NZ // 2 - 1:
                                               ZW * NZ // 2],
                                zero_t[:], op=MUL)
                        if t == NZ - 1:
                            fend = smal.tile([P, 1], F32, tag="fend")
                            nc.gpsimd.tensor_tensor(
                                fend[:], h_dst[:, L - 1:L], zero_t[:], op=MUL)
                if not last:
                    fences = [fmid, fend]
                    hx_list = hx_next
                if DEBUG and _rep == 0:
                    nc.sync.dma_start(dbg_h[blk][:], h_dst[:, 0:ZW])

            # ===================== head =================================
            if _rep == 0:
                # head-only params: emitted late so their DMA triggers queue
                # behind the main-loop transposes on the sync queue
                Whrep = _load_rep(nc, cst, Wh_d[:], C, "wh")
                Wfrep = _load_rep(nc, cst, Wf_d[:], DOUT, "wf")
                bf_s = cst.tile([P, 1], F32)
                nc.vector.memset(bf_s[:], 0.0)
                for b in range(B_LOC):
                    nc.sync.dma_start(
                        bf_s[C * b:C * b + DOUT, :], bf_d[:].unsqueeze(-1))
            if DEBUG and _rep == 0:
                nc.sync.dma_start(dbg_pool[:], pooled_parts[:])
            psum_ = smal.tile([P, 1], F32, tag="poolsum")
            nc.vector.reduce_sum(psum_[:], pooled_parts[:],
                                 axis=mybir.AxisListType.X)
            y_ps = tps.tile([P, C], F32, tag="tiny")
            for b in range(B_LOC):
                sl = slice(C * b, C * (b + 1))
                nc.tensor.matmul(
                    y_ps[sl, 0:1], Whrep[sl, :], psum_[sl, :],
                    tile_position=(C * b, C * b),
                )
            ybn = smal.tile([P, 1], F32, tag="ybn")
            nc.vector.tensor_scalar(
                ybn[:], y_ps[:, 0:1], svecL[:], tvec[:], op0=MUL, op1=ADD,
            )
            yg = smal.tile([P, 1], F32, tag="yg")
            nc.scalar.activation(yg[:], ybn[:], AF.Gelu)
            o_ps = tps.tile([P, C], F32, tag="tiny")
            for b in range(B_LOC):
                nc.tensor.matmul(
                    o_ps[C * b:C * b + DOUT, 0:1],
                    Wfrep[C * b:C * (b + 1), :],
                    yg[C * b:C * (b + 1), :],
                    tile_position=(C * b, C * b),
                )
            ob = smal.tile([P, 1], F32, tag="ob")
            for b in range(B_LOC):
                sl = slice(C * b, C * b + DOUT)
                nc.vector.tensor_tensor(ob[sl, :], o_ps[sl, 0:1], bf_s[sl, :],
                                        op=ADD)
            for b in range(B_LOC):
                nc.sync.dma_start(
                    out_d[b, :], ob[C * b:C * b + DOUT, 0],
                )

    _fix_sync_waits(nc)
    return nc


_NC_CACHE = [None]


def kernel(**inputs) -> np.ndarray:
    arrs = {k: np.asarray(v, dtype=np.float32) for k, v in inputs.items()}
    x = arrs["x"]
    B = x.shape[0]
    n_cores = 8
    bl = B // n_cores

    if _NC_CACHE[0] is None:
        _NC_CACHE[0] = build_program()
    nc = _NC_CACHE[0]

    params = {k: arrs[k] for k in (
        "fcW1", "fcb1", "fcW2", "fcb2", "Wh", "bh",
        "bn_gamma", "bn_beta", "bn_mean", "bn_var", "Wf", "bf")}
    in_maps = [
        {"x": np.ascontiguousarray(x[bl * i: bl * (i + 1)]), **params}
        for i in range(n_cores)
    ]
    res = run_bass_kernel_spmd(nc, in_maps, list(range(n_cores))).results
    return np.concatenate([res[i]["out"] for i in range(n_cores)], axis=0)


# revision 35
# speedup vs baseline: 1.5343x; 1.5343x over previous
"""Trainium2 Bass kernel for nn_CLFormer (3-block linear-attention transformer).

Sharding: pure data parallel — batch 32 split as 4 per NeuronCore across 8
cores; all parameters replicated; outputs concatenated.

Per-core layout: 4 batches x 32 channels on the 128 SBUF partitions
("channel-major" [128=4bx32c, L]). Per block:
  phase 1 (token-major): h slices are transposed SBUF->SBUF via the DMA
    xbar; E = exp(h) on ScalarE; the gram G = E^T h and the k-softmax
    denominator are fused into one PE matmul per chunk by extending the
    moving operand with a ones column (129 cols); q = E / rowsum(E) with
    the reciprocal on the DVE (reciprocal_approx_fast) and the multiply on
    GPSIMD; q is transposed back channel-major via the DMA xbar.
  phase 2 (channel-major): z1 = blockdiag(M1)^T q, gelu, z2 =
    blockdiag(W2)^T a1, gelu — one matmul per 512 tokens using
    block-diagonal stationaries (M1 = diag(1/ksum) G_masked W1 folds the
    attention output projection into FC1).
Activation LUT sets: Sqrt once at start, then per block Exp (phase 1) and
Gelu (phase 2); a fence tile makes each block's exps depend on the previous
block's last gelu so the scalar queue never interleaves the two sets.
"""
import sys
import numpy as np

for _p in ("/opt/trn_rl_repo", "/root/.axon_site/_ro/trn_rl_repo"):
    if _p not in sys.path:
        sys.path.append(_p)

from contextlib import ExitStack

import concourse.bass as bass
import concourse.mybir as mybir
import bass_rust
from concourse import tile
from concourse.masks import make_identity
from concourse.bass_utils import run_bass_kernel_spmd

F32 = mybir.dt.float32
BF16 = mybir.dt.bfloat16
AF = mybir.ActivationFunctionType
MUL = mybir.AluOpType.mult
ADD = mybir.AluOpType.add

P = 128
B_LOC = 4            # batches per core
C = 32               # channels
L = 16384            # sequence length
NB = 3               # transformer blocks
DOUT = 10
HEADS = 4
DH = 8
BN_EPS = 1e-5

SLC = 2048           # tokens per phase-1 slice
NSL = L // SLC       # 8 slices
NCH = SLC // 128     # 16 chunks per slice
ZW = 1024            # tokens per phase-2 chunk
NZ = L // ZW         # 16 chunks
EXT = 129            # used chunk width (128 h cols + 1 ones col)
PITCH = 144          # chunk pitch of the extended tile (xbar wants 16-elem mult)


# ---------------------------------------------------------------- waitfix --
_WF_SKIP = {"InstEventSemaphore"}
_wf_ctr = [0]


def _fix_sync_waits(nc):
    """Hoist excess sync waits onto InstEventSemaphore (this walrus build
    accepts only 1 wait per instruction). The event-sem executes on the same
    engine stream immediately before, preserving semantics."""
    for fn in nc.m.functions:
        new_blocks = []
        for blk in fn.blocks:
            out = []
            for ins in blk.instructions:
                tname = type(ins).__name__
                si = ins.sync_info
                if si is None or tname in _WF_SKIP:
                    out.append(ins)
                    continue
                waits = list(si.on_wait)
                if len(waits) <= 1:
                    out.append(ins)
                    continue
                keep = waits[-1:]
                excess = waits[:-1]
                for i in range(0, len(excess), 2):
                    chunk = excess[i:i + 2]
                    _wf_ctr[0] += 1
                    ev = mybir.InstEventSemaphore(
                        name=f"wfix{_wf_ctr[0]}", ins=[], outs=[])
                    ev.engine = ins.engine
                    ev.sync_info = mybir.SyncInfo(on_wait=chunk, on_update=[])
                    out.append(ev)
                ins.sync_info = mybir.SyncInfo(
                    on_wait=keep, on_update=list(si.on_update))
                out.append(ins)
            nb = bass_rust.BasicBlock(name=blk.name, instructions=out)
            new_blocks.append(nb)
        fn.blocks = new_blocks


# ---------------------------------------------------------------- program --
def _load_rep(nc, pool, src_ap, cols, tag, eng=None):
    """DRAM [32, cols] view -> SBUF [128, cols] f32, replicated across the 4
    batch partition strips (one DMA per strip)."""
    stage = pool.tile([P, cols], F32, tag=tag, name=tag)
    eng = eng if eng is not None else nc.sync
    for b in range(B_LOC):
        eng.dma_start(stage[C * b:C * (b + 1), :], src_ap)
    return stage


DEBUG = False


def build_program(reps=1):
    nc = bass.Bass()

    x_d = nc.declare_dram_parameter("x", [B_LOC, C, L], F32, isOutput=False)
    fcW1_d = nc.declare_dram_parameter("fcW1", [NB, C, C], F32, isOutput=False)
    fcb1_d = nc.declare_dram_parameter("fcb1", [NB, C], F32, isOutput=False)
    fcW2_d = nc.declare_dram_parameter("fcW2", [NB, C, C], F32, isOutput=False)
    fcb2_d = nc.declare_dram_parameter("fcb2", [NB, C], F32, isOutput=False)
    Wh_d = nc.declare_dram_parameter("Wh", [C, C], F32, isOutput=False)
    bh_d = nc.declare_dram_parameter("bh", [C], F32, isOutput=False)
    bng_d = nc.declare_dram_parameter("bn_gamma", [C], F32, isOutput=False)
    bnb_d = nc.declare_dram_parameter("bn_beta", [C], F32, isOutput=False)
    bnm_d = nc.declare_dram_parameter("bn_mean", [C], F32, isOutput=False)
    bnv_d = nc.declare_dram_parameter("bn_var", [C], F32, isOutput=False)
    Wf_d = nc.declare_dram_parameter("Wf", [C, DOUT], F32, isOutput=False)
    bf_d = nc.declare_dram_parameter("bf", [DOUT], F32, isOutput=False)
    out_d = nc.declare_dram_parameter("out", [B_LOC, DOUT], F32, isOutput=True)
    if DEBUG:
        dbg_G = [nc.declare_dram_parameter(f"dbg_G{i}", [P, EXT], F32,
                                           isOutput=True) for i in range(NB)]
        dbg_M1 = [nc.declare_dram_parameter(f"dbg_M1_{i}", [P, C], BF16,
                                            isOutput=True) for i in range(NB)]
        dbg_h = [nc.declare_dram_parameter(f"dbg_h{i}", [P, ZW], BF16,
                                           isOutput=True) for i in range(NB)]
        dbg_q = [nc.declare_dram_parameter(f"dbg_q{i}", [P, ZW], BF16,
                                           isOutput=True) for i in range(NB)]
        dbg_pool = nc.declare_dram_parameter("dbg_pool", [P, NZ], F32,
                                             isOutput=True)
        dbg_W1 = nc.declare_dram_parameter("dbg_W1", [P, NB * C], BF16,
                                           isOutput=True)
        dbg_ksC = [nc.declare_dram_parameter(f"dbg_ksC{i}", [P, 1], F32,
                                             isOutput=True) for i in range(NB)]
        dbg_S1 = [nc.declare_dram_parameter(f"dbg_S1_{i}", [P, P], BF16,
                                            isOutput=True) for i in range(NB)]

    with ExitStack() as ctx:
        tc = ctx.enter_context(tile.TileContext(nc))
        cst = ctx.enter_context(tc.tile_pool(name="cst", bufs=1))
        big = ctx.enter_context(tc.tile_pool(name="big", bufs=1))
        xst = ctx.enter_context(tc.tile_pool(name="xst", bufs=2))
        hexp = ctx.enter_context(tc.tile_pool(name="hexp", bufs=3))
        etp = ctx.enter_context(tc.tile_pool(name="etp", bufs=3))
        qtp = ctx.enter_context(tc.tile_pool(name="qtp", bufs=3))
        sqp = ctx.enter_context(tc.tile_pool(name="sqp", bufs=2))
        a1p = ctx.enter_context(tc.tile_pool(name="a1p", bufs=3))
        smal = ctx.enter_context(tc.tile_pool(name="smal", bufs=2))
        gps = ctx.enter_context(tc.tile_pool(name="gps", bufs=1, space="PSUM"))
        zp1 = ctx.enter_context(tc.tile_pool(name="zp1", bufs=2, space="PSUM"))
        zp2 = ctx.enter_context(tc.tile_pool(name="zp2", bufs=1, space="PSUM"))
        tps = ctx.enter_context(tc.tile_pool(name="tps", bufs=1, space="PSUM"))

        # ---- BN eval folding first: its Sqrt claims the scalar LUT before
        # the exp/gelu sets start cycling.
        # bn vectors ride the scalar HWDGE queue (idle at prologue)
        bh_r = _load_rep(nc, cst, bh_d[:].unsqueeze(-1), 1, "bh", nc.scalar)
        bng_r = _load_rep(nc, cst, bng_d[:].unsqueeze(-1), 1, "bng", nc.scalar)
        bnb_r = _load_rep(nc, cst, bnb_d[:].unsqueeze(-1), 1, "bnb", nc.scalar)
        bnm_r = _load_rep(nc, cst, bnm_d[:].unsqueeze(-1), 1, "bnm", nc.scalar)
        bnv_r = _load_rep(nc, cst, bnv_d[:].unsqueeze(-1), 1, "bnv", nc.scalar)
        eps_t = cst.tile([P, 1], F32)
        nc.vector.memset(eps_t[:], BN_EPS)
        sq_t = cst.tile([P, 1], F32)
        nc.scalar.activation(sq_t[:], bnv_r[:], AF.Sqrt, bias=eps_t[:])
        rs_t = cst.tile([P, 1], F32)
        nc.vector.reciprocal(rs_t[:], sq_t[:])
        svec = cst.tile([P, 1], F32)
        nc.vector.tensor_tensor(svec[:], rs_t[:], bng_r[:], op=MUL)
        svecL = cst.tile([P, 1], F32)
        nc.vector.tensor_scalar_mul(svecL[:], svec[:], 1.0 / L)
        t0 = cst.tile([P, 1], F32)
        nc.vector.tensor_tensor(t0[:], bh_r[:], bnm_r[:],
                                op=mybir.AluOpType.subtract)
        t1 = cst.tile([P, 1], F32)
        nc.vector.tensor_tensor(t1[:], t0[:], svec[:], op=MUL)
        tvec = cst.tile([P, 1], F32)
        nc.vector.tensor_tensor(tvec[:], t1[:], bnb_r[:], op=ADD)

        # ---- constants -------------------------------------------------
        ident = cst.tile([P, P], BF16)
        make_identity(nc, ident[:])
        headmask = cst.tile([P, P], BF16)
        nc.vector.memset(headmask[:], 1.0)
        hm_v = headmask[:].rearrange("p (g i) -> p g i", i=DH)
        nc.gpsimd.affine_select(
            out=hm_v, in_=hm_v, pattern=[[-DH, P // DH], [0, DH]],
            compare_op=mybir.AluOpType.is_ge, fill=0.0,
            base=0, channel_multiplier=1)
        nc.gpsimd.affine_select(
            out=hm_v, in_=hm_v, pattern=[[DH, P // DH], [0, DH]],
            compare_op=mybir.AluOpType.is_ge, fill=0.0,
            base=DH - 1, channel_multiplier=-1)

        # ---- params: per-strip DMAs, spread across idle trigger queues ----
        def _load_w(src_d, tag, eng):
            stage = cst.tile([P, NB * C], F32, tag=f"{tag}st", name=f"{tag}st")
            dst = stage[:].rearrange("(b c) (n k) -> b c n k", b=B_LOC, n=NB)
            for b in range(B_LOC):
                eng.dma_start(dst[b], src_d[:].rearrange("n c k -> c n k"))
            rep = cst.tile([P, NB * C], BF16, tag=f"{tag}bf", name=f"{tag}bf")
            nc.vector.tensor_copy(rep[:], stage[:])
            return rep

        def _load_b(src_d, tag, eng):
            stage = cst.tile([P, NB], F32, tag=tag, name=tag)
            for b in range(B_LOC):
                eng.dma_start(stage[C * b:C * (b + 1), :],
                              src_d[:].rearrange("n c -> c n"))
            return stage

        # W1/b1 (needed at the first M1 build) on the scalar queue right
        # after the bn vectors; W2/b2 (needed at the first phase 2) on the
        # gpsimd SWDGE queue, idle during the prologue
        W1rep = _load_w(fcW1_d, "w1", nc.scalar)
        b1rep = _load_b(fcb1_d, "b1", nc.scalar)
        W2rep = _load_w(fcW2_d, "w2", nc.gpsimd)
        b2rep = _load_b(fcb2_d, "b2", nc.gpsimd)
        # block-diagonal W2 stationaries
        W2diag = []
        for i in range(NB):
            wd = cst.tile([P, P], BF16, tag=f"w2d{i}", name=f"w2d{i}")
            nc.vector.memset(wd[:], 0.0)
            for b in range(B_LOC):
                sl = slice(C * b, C * (b + 1))
                nc.vector.tensor_copy(
                    wd[sl, C * b:C * (b + 1)], W2rep[sl, C * i:C * (i + 1)])
            W2diag.append(wd)
        zero_t = cst.tile([P, 1], F32)
        nc.vector.memset(zero_t[:], 0.0)
        ones_t = cst.tile([P, 1], F32)
        nc.vector.memset(ones_t[:], 1.0)

        # (repetition loop for benchmarking only; reps=1 in production)
        for _rep in range(reps):
            # two persistent channel-major h buffers, ping-ponged per block
            h_tiles = [
                big.tile([P, L], BF16, tag=f"h{i}_{_rep}", name=f"h{i}_{_rep}")
                for i in range(2)
            ]
            q_cm = big.tile([P, L], BF16, tag=f"qcm_{_rep}", name=f"qcm_{_rep}")
            pooled_parts = cst.tile([P, NZ], F32, tag=f"pool_{_rep}",
                                    name=f"pool_{_rep}")

            # ---- input: load x slices, cast to bf16 on the DVE ---------
            # high priority so the x pipeline beats the param-load DMA
            # triggers queued ahead of it on the sync queue
            # x loads first on the sync queue (boosted above everything else
            # queued there); casts on the otherwise-idle DVE
            x_cm = x_d[:].rearrange("b c l -> (b c) l")
            with tc.high_priority():
                for s in range(NSL):
                    xs = xst.tile([P, SLC], F32, tag="xs", bufs=3)
                    nc.sync.dma_start(xs[:], x_cm[:, SLC * s:SLC * (s + 1)])
                    nc.vector.tensor_copy(
                        h_tiles[0][:, SLC * s:SLC * (s + 1)], xs[:])

            def _new_hx(src_tile, s):
                hx = hexp.tile([P, NCH, PITCH], BF16, tag="hex", bufs=8,
                               name="hx")
                nc.vector.memset(hx[:, :, P:EXT], 1.0)
                nc.sync.dma_start_transpose(
                    out=hx[:, :, 0:P],
                    in_=src_tile[:, SLC * s:SLC * (s + 1)],
                )
                return hx

            fences = [zero_t, zero_t]
            hx_list = None
            for blk in range(NB):
                h_src = h_tiles[blk % 2]
                h_dst = h_tiles[(blk + 1) % 2]
                G_ps = gps.tile([P, EXT], F32, tag="G")

                # ===================== phase 1 (token-major) ============
                if blk == 0:
                    hx_list = [_new_hx(h_src, s) for s in range(NSL)]
                for s in range(NSL):
                    hx = hx_list[s]
                    et = etp.tile([P, SLC], BF16, tag="et")
                    nc.scalar.activation(
                        et[:].rearrange("p (c l) -> p c l", l=P),
                        hx[:, :, 0:P], AF.Exp,
                        bias=fences[0] if s < NSL // 2 else fences[1])
                    # fused gram + ksum: col 128 of the moving operand is 1
                    for c in range(NCH):
                        nc.tensor.matmul(
                            G_ps[:],
                            et[:, P * c:P * (c + 1)],
                            hx[:, c, 0:EXT],
                            start=(s == 0 and c == 0),
                            stop=(s == NSL - 1 and c == NCH - 1),
                        )
                    # q = E / rowsum_d(E)
                    sq = sqp.tile([P, SLC // DH], F32, tag="sq")
                    nc.vector.reduce_sum(
                        sq[:],
                        et[:].rearrange("p (g d) -> p g d", d=DH),
                        axis=mybir.AxisListType.X,
                    )
                    rq = sqp.tile([P, SLC // DH], F32, tag="rq")
                    nc.vector.reciprocal(rq[:], sq[:])
                    qt = qtp.tile([P, SLC], BF16, tag="qt")
                    et_v = et[:].rearrange("p (g d) -> p g d", d=DH)
                    qt_v = qt[:].rearrange("p (g d) -> p g d", d=DH)
                    rq_v = rq[:].unsqueeze(-1)
                    nc.gpsimd.tensor_tensor(
                        qt_v[:], et_v[:],
                        rq_v[:].broadcast_to([P, SLC // DH, DH]),
                        op=MUL,
                    )
                    nc.sync.dma_start_transpose(
                        out=q_cm[:, SLC * s:SLC * (s + 1)]
                        .rearrange("p (c l) -> p c l", l=P),
                        in_=qt[:],
                    )

                # ===================== M1 build =========================
                ksC = smal.tile([P, 1], F32, tag="ksC")
                nc.vector.reciprocal(ksC[:], G_ps[:, P:EXT])
                G_sb = smal.tile([P, P], BF16, tag="Gsb")
                nc.vector.tensor_tensor(G_sb[:], G_ps[:, 0:P], headmask[:],
                                        op=MUL)
                GT_ps = tps.tile([P, P], F32, tag="tiny")
                nc.tensor.matmul(GT_ps[:], G_sb[:], ident[:])
                GT_sb = smal.tile([P, P], BF16, tag="gtsb")
                nc.vector.tensor_copy(GT_sb[:], GT_ps[:])
                M1u_ps = tps.tile([P, C], F32, tag="tiny")
                for b in range(B_LOC):
                    sl = slice(C * b, C * (b + 1))
                    nc.tensor.matmul(
                        M1u_ps[sl, 0:C], GT_sb[sl, C * b:C * (b + 1)],
                        W1rep[sl, C * blk:C * (blk + 1)],
                        tile_position=(C * b, C * b),
                    )
                M1 = smal.tile([P, C], BF16, tag="m1")
                nc.vector.tensor_scalar_mul(M1[:], M1u_ps[:], ksC[:])
                if DEBUG and _rep == 0:
                    Gf = smal.tile([P, EXT], F32, tag="dbgG")
                    nc.vector.tensor_copy(Gf[:], G_ps[:])
                    nc.sync.dma_start(dbg_G[blk][:], Gf[:])
                    nc.sync.dma_start(dbg_M1[blk][:], M1[:])
                    nc.sync.dma_start(dbg_q[blk][:], q_cm[:, 0:ZW])
                    nc.sync.dma_start(dbg_ksC[blk][:], ksC[:])
                    if blk == 0:
                        nc.sync.dma_start(dbg_W1[:], W1rep[:])
                S1 = smal.tile([P, P], BF16, tag="s1")
                nc.vector.memset(S1[:], 0.0)
                for b in range(B_LOC):
                    sl = slice(C * b, C * (b + 1))
                    nc.vector.tensor_copy(S1[sl, C * b:C * (b + 1)], M1[sl, :])
                if DEBUG and _rep == 0:
                    nc.sync.dma_start(dbg_S1[blk][:], S1[:])

                # ===================== phase 2 (channel-major) ==========
                last = blk == NB - 1
                hx_next = []
                for t in range(NZ):
                    z1 = zp1.tile([P, ZW], F32, tag="z1")
                    for hh in range(ZW // 512):
                        nc.tensor.matmul(
                            z1[:, 512 * hh:512 * (hh + 1)], S1[:],
                            q_cm[:, ZW * t + 512 * hh:ZW * t + 512 * (hh + 1)],
                        )
                    a1 = a1p.tile([P, ZW], BF16, tag="a1")
                    nc.scalar.activation(a1[:], z1[:], AF.Gelu,
                                         bias=b1rep[:, blk:blk + 1])
                    z2 = zp2.tile([P, ZW], F32, tag="z2")
                    for hh in range(ZW // 512):
                        nc.tensor.matmul(
                            z2[:, 512 * hh:512 * (hh + 1)], W2diag[blk][:],
                            a1[:, 512 * hh:512 * (hh + 1)],
                        )
                    if last:
                        nc.scalar.activation(
                            h_dst[:, ZW * t:ZW * (t + 1)], z2[:],
                            AF.Gelu, bias=b2rep[:, blk:blk + 1],
                            accum_out=pooled_parts[:, t:t + 1],
                        )
                    else:
                        nc.scalar.activation(
                            h_dst[:, ZW * t:ZW * (t + 1)], z2[:],
                            AF.Gelu, bias=b2rep[:, blk:blk + 1],
                        )
                        # next block's token-major transpose for slice t//2
                        # fires as soon as its two chunks are written
                        if t % 2 == 1:
                            hx_next.append(_new_hx(h_dst, t // 2))
                        # mid fence: first half of next block's exps may
                        # start once chunks 0..7 are written
                        if t == NZ // 2 - 1:
                            fmid = smal.tile([P, 1], F32, tag="fmid")
                            nc.gpsimd.tensor_tensor(
                                fmid[:], h_dst[:, ZW * NZ // 2 - 1:
                                               ZW * NZ // 2],
                                zero_t[:], op=MUL)
                        if t == NZ - 1:
                            fend = smal.tile([P, 1], F32, tag="fend")
                            nc.gpsimd.tensor_tensor(
                                fend[:], h_dst[:, L - 1:L], zero_t[:], op=MUL)
                if not last:
                    fences = [fmid, fend]
                    hx_list = hx_next
                if DEBUG and _rep == 0:
                    nc.sync.dma_start(dbg_h[blk][:], h_dst[:, 0:ZW])

            # ===================== head =================================
            if _rep == 0:
                # head-only params: emitted late so their DMA triggers queue
                # behind the main-loop transposes on the sync queue
                Whrep = _load_rep(nc, cst, Wh_d[:], C, "wh")
                Wfrep = _load_rep(nc, cst, Wf_d[:], DOUT, "wf")
                bf_s = cst.tile([P, 1], F32)
                nc.vector.memset(bf_s[:], 0.0)
                for b in range(B_LOC):
                    nc.sync.dma_start(
                        bf_s[C * b:C * b + DOUT, :], bf_d[:].unsqueeze(-1))
            if DEBUG and _rep == 0:
                nc.sync.dma_start(dbg_pool[:], pooled_parts[:])
            psum_ = smal.tile([P, 1], F32, tag="poolsum")
            nc.vector.reduce_sum(psum_[:], pooled_parts[:],
                                 axis=mybir.AxisListType.X)
            y_ps = tps.tile([P, C], F32, tag="tiny")
            for b in range(B_LOC):
                sl = slice(C * b, C * (b + 1))
                nc.tensor.matmul(
                    y_ps[sl, 0:1], Whrep[sl, :], psum_[sl, :],
                    tile_position=(C * b, C * b),
                )
            ybn = smal.tile([P, 1], F32, tag="ybn")
            nc.vector.tensor_scalar(
                ybn[:], y_ps[:, 0:1], svecL[:], tvec[:], op0=MUL, op1=ADD,
            )
            yg = smal.tile([P, 1], F32, tag="yg")
            nc.scalar.activation(yg[:], ybn[:], AF.Gelu)
            o_ps = tps.tile([P, C], F32, tag="tiny")
            for b in range(B_LOC):
                nc.tensor.matmul(
                    o_ps[C * b:C * b + DOUT, 0:1],
                    Wfrep[C * b:C * (b + 1), :],
                    yg[C * b:C * (b + 1), :],
                    tile_position=(C * b, C * b),
                )
            ob = smal.tile([P, 1], F32, tag="ob")
            for b in range(B_LOC):
                sl = slice(C * b, C * b + DOUT)
                nc.vector.tensor_tensor(ob[sl, :], o_ps[sl, 0:1], bf_s[sl, :],
                                        op=ADD)
            for b in range(B_LOC):
                nc.sync.dma_start(
                    out_d[b, :], ob[C * b:C * b + DOUT, 0],
                )

    _fix_sync_waits(nc)
    return nc


_NC_CACHE = [None]


def kernel(**inputs) -> np.ndarray:
    arrs = {k: np.asarray(v, dtype=np.float32) for k, v in inputs.items()}
    x = arrs["x"]
    B = x.shape[0]
    n_cores = 8
    bl = B // n_cores

    if _NC_CACHE[0] is None:
        _NC_CACHE[0] = build_program()
    nc = _NC_CACHE[0]

    params = {k: arrs[k] for k in (
        "fcW1", "fcb1", "fcW2", "fcb2", "Wh", "bh",
        "bn_gamma", "bn_beta", "bn_mean", "bn_var", "Wf", "bf")}
    in_maps = [
        {"x": np.ascontiguousarray(x[bl * i: bl * (i + 1)]), **params}
        for i in range(n_cores)
    ]
    res = run_bass_kernel_spmd(nc, in_maps, list(range(n_cores))).results
    return np.concatenate([res[i]["out"] for i in range(n_cores)], axis=0)


# revision 37
# speedup vs baseline: 1.5418x; 1.0049x over previous
"""Trainium2 Bass kernel for nn_CLFormer (3-block linear-attention transformer).

Sharding: pure data parallel — batch 32 split as 4 per NeuronCore across 8
cores; all parameters replicated; outputs concatenated.

Per-core layout: 4 batches x 32 channels on the 128 SBUF partitions
("channel-major" [128=4bx32c, L]). Per block:
  phase 1 (token-major): h slices are transposed SBUF->SBUF via the DMA
    xbar; E = exp(h) on ScalarE; the gram G = E^T h and the k-softmax
    denominator are fused into one PE matmul per chunk by extending the
    moving operand with a ones column (129 cols); q = E / rowsum(E) with
    the reciprocal on the DVE (reciprocal_approx_fast) and the multiply on
    GPSIMD; q is transposed back channel-major via the DMA xbar.
  phase 2 (channel-major): z1 = blockdiag(M1)^T q, gelu, z2 =
    blockdiag(W2)^T a1, gelu — one matmul per 512 tokens using
    block-diagonal stationaries (M1 = diag(1/ksum) G_masked W1 folds the
    attention output projection into FC1).
Activation LUT sets: Sqrt once at start, then per block Exp (phase 1) and
Gelu (phase 2); a fence tile makes each block's exps depend on the previous
block's last gelu so the scalar queue never interleaves the two sets.
"""
import sys
import numpy as np

for _p in ("/opt/trn_rl_repo", "/root/.axon_site/_ro/trn_rl_repo"):
    if _p not in sys.path:
        sys.path.append(_p)

from contextlib import ExitStack

import concourse.bass as bass
import concourse.mybir as mybir
import bass_rust
from concourse import tile
from concourse.masks import make_identity
from concourse.bass_utils import run_bass_kernel_spmd

F32 = mybir.dt.float32
BF16 = mybir.dt.bfloat16
AF = mybir.ActivationFunctionType
MUL = mybir.AluOpType.mult
ADD = mybir.AluOpType.add

P = 128
B_LOC = 4            # batches per core
C = 32               # channels
L = 16384            # sequence length
NB = 3               # transformer blocks
DOUT = 10
HEADS = 4
DH = 8
BN_EPS = 1e-5

SLC = 2048           # tokens per phase-1 slice
NSL = L // SLC       # 8 slices
NCH = SLC // 128     # 16 chunks per slice
ZW = 1024            # tokens per phase-2 chunk
NZ = L // ZW         # 16 chunks
EXT = 129            # used chunk width (128 h cols + 1 ones col)
PITCH = 144          # chunk pitch of the extended tile (xbar wants 16-elem mult)


# ---------------------------------------------------------------- waitfix --
_WF_SKIP = {"InstEventSemaphore"}
_wf_ctr = [0]


def _fix_sync_waits(nc):
    """Hoist excess sync waits onto InstEventSemaphore (this walrus build
    accepts only 1 wait per instruction). The event-sem executes on the same
    engine stream immediately before, preserving semantics."""
    for fn in nc.m.functions:
        new_blocks = []
        for blk in fn.blocks:
            out = []
            for ins in blk.instructions:
                tname = type(ins).__name__
                si = ins.sync_info
                if si is None or tname in _WF_SKIP:
                    out.append(ins)
                    continue
                waits = list(si.on_wait)
                if len(waits) <= 1:
                    out.append(ins)
                    continue
                keep = waits[-1:]
                excess = waits[:-1]
                for i in range(0, len(excess), 2):
                    chunk = excess[i:i + 2]
                    _wf_ctr[0] += 1
                    ev = mybir.InstEventSemaphore(
                        name=f"wfix{_wf_ctr[0]}", ins=[], outs=[])
                    ev.engine = ins.engine
                    ev.sync_info = mybir.SyncInfo(on_wait=chunk, on_update=[])
                    out.append(ev)
                ins.sync_info = mybir.SyncInfo(
                    on_wait=keep, on_update=list(si.on_update))
                out.append(ins)
            nb = bass_rust.BasicBlock(name=blk.name, instructions=out)
            new_blocks.append(nb)
        fn.blocks = new_blocks


# ---------------------------------------------------------------- program --
def _load_rep(nc, pool, src_ap, cols, tag, eng=None):
    """DRAM [32, cols] view -> SBUF [128, cols] f32, replicated across the 4
    batch partition strips (one DMA per strip)."""
    stage = pool.tile([P, cols], F32, tag=tag, name=tag)
    eng = eng if eng is not None else nc.sync
    for b in range(B_LOC):
        eng.dma_start(stage[C * b:C * (b + 1), :], src_ap)
    return stage


DEBUG = False


def build_program(reps=1):
    nc = bass.Bass()

    x_d = nc.declare_dram_parameter("x", [B_LOC, C, L], F32, isOutput=False)
    fcW1_d = nc.declare_dram_parameter("fcW1", [NB, C, C], F32, isOutput=False)
    fcb1_d = nc.declare_dram_parameter("fcb1", [NB, C], F32, isOutput=False)
    fcW2_d = nc.declare_dram_parameter("fcW2", [NB, C, C], F32, isOutput=False)
    fcb2_d = nc.declare_dram_parameter("fcb2", [NB, C], F32, isOutput=False)
    Wh_d = nc.declare_dram_parameter("Wh", [C, C], F32, isOutput=False)
    bh_d = nc.declare_dram_parameter("bh", [C], F32, isOutput=False)
    bng_d = nc.declare_dram_parameter("bn_gamma", [C], F32, isOutput=False)
    bnb_d = nc.declare_dram_parameter("bn_beta", [C], F32, isOutput=False)
    bnm_d = nc.declare_dram_parameter("bn_mean", [C], F32, isOutput=False)
    bnv_d = nc.declare_dram_parameter("bn_var", [C], F32, isOutput=False)
    Wf_d = nc.declare_dram_parameter("Wf", [C, DOUT], F32, isOutput=False)
    bf_d = nc.declare_dram_parameter("bf", [DOUT], F32, isOutput=False)
    out_d = nc.declare_dram_parameter("out", [B_LOC, DOUT], F32, isOutput=True)
    if DEBUG:
        dbg_G = [nc.declare_dram_parameter(f"dbg_G{i}", [P, EXT], F32,
                                           isOutput=True) for i in range(NB)]
        dbg_M1 = [nc.declare_dram_parameter(f"dbg_M1_{i}", [P, C], BF16,
                                            isOutput=True) for i in range(NB)]
        dbg_h = [nc.declare_dram_parameter(f"dbg_h{i}", [P, ZW], BF16,
                                           isOutput=True) for i in range(NB)]
        dbg_q = [nc.declare_dram_parameter(f"dbg_q{i}", [P, ZW], BF16,
                                           isOutput=True) for i in range(NB)]
        dbg_pool = nc.declare_dram_parameter("dbg_pool", [P, NZ], F32,
                                             isOutput=True)
        dbg_W1 = nc.declare_dram_parameter("dbg_W1", [P, NB * C], BF16,
                                           isOutput=True)
        dbg_ksC = [nc.declare_dram_parameter(f"dbg_ksC{i}", [P, 1], F32,
                                             isOutput=True) for i in range(NB)]
        dbg_S1 = [nc.declare_dram_parameter(f"dbg_S1_{i}", [P, P], BF16,
                                            isOutput=True) for i in range(NB)]

    with ExitStack() as ctx:
        tc = ctx.enter_context(tile.TileContext(nc))
        cst = ctx.enter_context(tc.tile_pool(name="cst", bufs=1))
        big = ctx.enter_context(tc.tile_pool(name="big", bufs=1))
        xst = ctx.enter_context(tc.tile_pool(name="xst", bufs=2))
        hexp = ctx.enter_context(tc.tile_pool(name="hexp", bufs=3))
        etp = ctx.enter_context(tc.tile_pool(name="etp", bufs=3))
        qtp = ctx.enter_context(tc.tile_pool(name="qtp", bufs=3))
        sqp = ctx.enter_context(tc.tile_pool(name="sqp", bufs=2))
        a1p = ctx.enter_context(tc.tile_pool(name="a1p", bufs=3))
        smal = ctx.enter_context(tc.tile_pool(name="smal", bufs=2))
        gps = ctx.enter_context(tc.tile_pool(name="gps", bufs=1, space="PSUM"))
        zp1 = ctx.enter_context(tc.tile_pool(name="zp1", bufs=2, space="PSUM"))
        zp2 = ctx.enter_context(tc.tile_pool(name="zp2", bufs=1, space="PSUM"))
        tps = ctx.enter_context(tc.tile_pool(name="tps", bufs=1, space="PSUM"))

        # ---- BN eval folding first: its Sqrt claims the scalar LUT before
        # the exp/gelu sets start cycling.
        # bn vectors ride the scalar HWDGE queue (idle at prologue)
        bh_r = _load_rep(nc, cst, bh_d[:].unsqueeze(-1), 1, "bh", nc.scalar)
        bng_r = _load_rep(nc, cst, bng_d[:].unsqueeze(-1), 1, "bng", nc.scalar)
        bnb_r = _load_rep(nc, cst, bnb_d[:].unsqueeze(-1), 1, "bnb", nc.scalar)
        bnm_r = _load_rep(nc, cst, bnm_d[:].unsqueeze(-1), 1, "bnm", nc.scalar)
        bnv_r = _load_rep(nc, cst, bnv_d[:].unsqueeze(-1), 1, "bnv", nc.scalar)
        eps_t = cst.tile([P, 1], F32)
        nc.vector.memset(eps_t[:], BN_EPS)
        sq_t = cst.tile([P, 1], F32)
        nc.scalar.activation(sq_t[:], bnv_r[:], AF.Sqrt, bias=eps_t[:])
        rs_t = cst.tile([P, 1], F32)
        nc.vector.reciprocal(rs_t[:], sq_t[:])
        svec = cst.tile([P, 1], F32)
        nc.vector.tensor_tensor(svec[:], rs_t[:], bng_r[:], op=MUL)
        svecL = cst.tile([P, 1], F32)
        nc.vector.tensor_scalar_mul(svecL[:], svec[:], 1.0 / L)
        t0 = cst.tile([P, 1], F32)
        nc.vector.tensor_tensor(t0[:], bh_r[:], bnm_r[:],
                                op=mybir.AluOpType.subtract)
        t1 = cst.tile([P, 1], F32)
        nc.vector.tensor_tensor(t1[:], t0[:], svec[:], op=MUL)
        tvec = cst.tile([P, 1], F32)
        nc.vector.tensor_tensor(tvec[:], t1[:], bnb_r[:], op=ADD)

        # ---- constants -------------------------------------------------
        ident = cst.tile([P, P], BF16)
        make_identity(nc, ident[:])
        headmask = cst.tile([P, P], BF16)
        nc.vector.memset(headmask[:], 1.0)
        hm_v = headmask[:].rearrange("p (g i) -> p g i", i=DH)
        nc.gpsimd.affine_select(
            out=hm_v, in_=hm_v, pattern=[[-DH, P // DH], [0, DH]],
            compare_op=mybir.AluOpType.is_ge, fill=0.0,
            base=0, channel_multiplier=1)
        nc.gpsimd.affine_select(
            out=hm_v, in_=hm_v, pattern=[[DH, P // DH], [0, DH]],
            compare_op=mybir.AluOpType.is_ge, fill=0.0,
            base=DH - 1, channel_multiplier=-1)

        # ---- params: per-strip DMAs, spread across idle trigger queues ----
        def _load_w(src_d, tag, eng):
            stage = cst.tile([P, NB * C], F32, tag=f"{tag}st", name=f"{tag}st")
            dst = stage[:].rearrange("(b c) (n k) -> b c n k", b=B_LOC, n=NB)
            for b in range(B_LOC):
                eng.dma_start(dst[b], src_d[:].rearrange("n c k -> c n k"))
            rep = cst.tile([P, NB * C], BF16, tag=f"{tag}bf", name=f"{tag}bf")
            nc.vector.tensor_copy(rep[:], stage[:])
            return rep

        def _load_b(src_d, tag, eng):
            stage = cst.tile([P, NB], F32, tag=tag, name=tag)
            for b in range(B_LOC):
                eng.dma_start(stage[C * b:C * (b + 1), :],
                              src_d[:].rearrange("n c -> c n"))
            return stage

        # W1/b1 (needed at the first M1 build) on the scalar queue right
        # after the bn vectors; W2/b2 (needed at the first phase 2) on the
        # gpsimd SWDGE queue, idle during the prologue
        W1rep = _load_w(fcW1_d, "w1", nc.scalar)
        b1rep = _load_b(fcb1_d, "b1", nc.scalar)
        W2rep = _load_w(fcW2_d, "w2", nc.gpsimd)
        b2rep = _load_b(fcb2_d, "b2", nc.gpsimd)
        # block-diagonal W2 stationaries
        W2diag = []
        for i in range(NB):
            wd = cst.tile([P, P], BF16, tag=f"w2d{i}", name=f"w2d{i}")
            nc.vector.memset(wd[:], 0.0)
            for b in range(B_LOC):
                sl = slice(C * b, C * (b + 1))
                nc.vector.tensor_copy(
                    wd[sl, C * b:C * (b + 1)], W2rep[sl, C * i:C * (i + 1)])
            W2diag.append(wd)
        zero_t = cst.tile([P, 1], F32)
        nc.vector.memset(zero_t[:], 0.0)
        ones_t = cst.tile([P, 1], F32)
        nc.vector.memset(ones_t[:], 1.0)

        # (repetition loop for benchmarking only; reps=1 in production)
        for _rep in range(reps):
            # two persistent channel-major h buffers, ping-ponged per block
            h_tiles = [
                big.tile([P, L], BF16, tag=f"h{i}_{_rep}", name=f"h{i}_{_rep}")
                for i in range(2)
            ]
            q_cm = big.tile([P, L], BF16, tag=f"qcm_{_rep}", name=f"qcm_{_rep}")
            pooled_parts = cst.tile([P, NZ], F32, tag=f"pool_{_rep}",
                                    name=f"pool_{_rep}")

            # ---- input: load x slices, cast to bf16 on the DVE ---------
            # high priority so the x pipeline beats the param-load DMA
            # triggers queued ahead of it on the sync queue
            # x loads first on the sync queue (boosted above everything else
            # queued there); casts on the otherwise-idle DVE
            x_cm = x_d[:].rearrange("b c l -> (b c) l")
            with tc.high_priority():
                for s in range(NSL):
                    xs = xst.tile([P, SLC], F32, tag="xs", bufs=3)
                    nc.sync.dma_start(xs[:], x_cm[:, SLC * s:SLC * (s + 1)])
                    nc.vector.tensor_copy(
                        h_tiles[0][:, SLC * s:SLC * (s + 1)], xs[:])

            def _new_hx(src_tile, s):
                hx = hexp.tile([P, NCH, PITCH], BF16, tag="hex", bufs=8,
                               name="hx")
                nc.vector.memset(hx[:, :, P:EXT], 1.0)
                nc.sync.dma_start_transpose(
                    out=hx[:, :, 0:P],
                    in_=src_tile[:, SLC * s:SLC * (s + 1)],
                )
                return hx

            def _p1_slice(blk, s, hx, G_ps):
                """Phase-1 work for one token-major slice: exp, fused
                gram+ksum matmuls, q normalization, q transpose."""
                et = etp.tile([P, SLC], BF16, tag="et", name="et")
                nc.scalar.activation(
                    et[:].rearrange("p (c l) -> p c l", l=P),
                    hx[:, :, 0:P], AF.Exp)
                # fused gram + ksum: col 128 of the moving operand is 1
                for c in range(NCH):
                    nc.tensor.matmul(
                        G_ps[:],
                        et[:, P * c:P * (c + 1)],
                        hx[:, c, 0:EXT],
                        start=(s == 0 and c == 0),
                        stop=(s == NSL - 1 and c == NCH - 1),
                    )
                # q = E / rowsum_d(E)
                sq = sqp.tile([P, SLC // DH], F32, tag="sq", name="sq")
                nc.vector.reduce_sum(
                    sq[:],
                    et[:].rearrange("p (g d) -> p g d", d=DH),
                    axis=mybir.AxisListType.X,
                )
                rq = sqp.tile([P, SLC // DH], F32, tag="rq", name="rq")
                nc.vector.reciprocal(rq[:], sq[:])
                qt = qtp.tile([P, SLC], BF16, tag="qt", name="qt")
                et_v = et[:].rearrange("p (g d) -> p g d", d=DH)
                qt_v = qt[:].rearrange("p (g d) -> p g d", d=DH)
                rq_v = rq[:].unsqueeze(-1)
                nc.gpsimd.tensor_tensor(
                    qt_v[:], et_v[:],
                    rq_v[:].broadcast_to([P, SLC // DH, DH]),
                    op=MUL,
                )
                nc.sync.dma_start_transpose(
                    out=q_cm[:, SLC * s:SLC * (s + 1)]
                    .rearrange("p (c l) -> p c l", l=P),
                    in_=qt[:],
                )

            # block 0's phase 1 runs standalone; later blocks' phase-1
            # slices are emitted inline inside the previous block's phase 2
            # so the in-order engine queues interleave them correctly
            hx0 = [_new_hx(h_tiles[0], s) for s in range(NSL)]
            G_cur = gps.tile([P, EXT], F32, tag="G", name="G0")
            for s in range(NSL):
                _p1_slice(0, s, hx0[s], G_cur)

            for blk in range(NB):
                h_src = h_tiles[blk % 2]
                h_dst = h_tiles[(blk + 1) % 2]
                G_ps = G_cur

                # ===================== M1 build =========================
                ksC = smal.tile([P, 1], F32, tag="ksC")
                nc.vector.reciprocal(ksC[:], G_ps[:, P:EXT])
                G_sb = smal.tile([P, P], BF16, tag="Gsb")
                nc.vector.tensor_tensor(G_sb[:], G_ps[:, 0:P], headmask[:],
                                        op=MUL)
                GT_ps = tps.tile([P, P], F32, tag="tiny")
                nc.tensor.matmul(GT_ps[:], G_sb[:], ident[:])
                GT_sb = smal.tile([P, P], BF16, tag="gtsb")
                nc.vector.tensor_copy(GT_sb[:], GT_ps[:])
                M1u_ps = tps.tile([P, C], F32, tag="tiny")
                for b in range(B_LOC):
                    sl = slice(C * b, C * (b + 1))
                    nc.tensor.matmul(
                        M1u_ps[sl, 0:C], GT_sb[sl, C * b:C * (b + 1)],
                        W1rep[sl, C * blk:C * (blk + 1)],
                        tile_position=(C * b, C * b),
                    )
                M1 = smal.tile([P, C], BF16, tag="m1")
                nc.vector.tensor_scalar_mul(M1[:], M1u_ps[:], ksC[:])
                if DEBUG and _rep == 0:
                    Gf = smal.tile([P, EXT], F32, tag="dbgG")
                    nc.vector.tensor_copy(Gf[:], G_ps[:])
                    nc.sync.dma_start(dbg_G[blk][:], Gf[:])
                    nc.sync.dma_start(dbg_M1[blk][:], M1[:])
                    nc.sync.dma_start(dbg_q[blk][:], q_cm[:, 0:ZW])
                    nc.sync.dma_start(dbg_ksC[blk][:], ksC[:])
                    if blk == 0:
                        nc.sync.dma_start(dbg_W1[:], W1rep[:])
                S1 = smal.tile([P, P], BF16, tag="s1")
                nc.vector.memset(S1[:], 0.0)
                for b in range(B_LOC):
                    sl = slice(C * b, C * (b + 1))
                    nc.vector.tensor_copy(S1[sl, C * b:C * (b + 1)], M1[sl, :])
                if DEBUG and _rep == 0:
                    nc.sync.dma_start(dbg_S1[blk][:], S1[:])

                # ===================== phase 2 (channel-major) ==========
                # next block's phase-1 slices are emitted inline: the first
                # four right after chunk 7 (their h chunks 0..7 are written
                # by then), the rest after the loop
                last = blk == NB - 1
                hx_next = []
                if not last:
                    G_next = gps.tile([P, EXT], F32, tag="G", name="Gn")
                for t in range(NZ):
                    z1 = zp1.tile([P, ZW], F32, tag="z1")
                    for hh in range(ZW // 512):
                        nc.tensor.matmul(
                            z1[:, 512 * hh:512 * (hh + 1)], S1[:],
                            q_cm[:, ZW * t + 512 * hh:ZW * t + 512 * (hh + 1)],
                        )
                    a1 = a1p.tile([P, ZW], BF16, tag="a1")
                    nc.scalar.activation(a1[:], z1[:], AF.Gelu,
                                         bias=b1rep[:, blk:blk + 1])
                    z2 = zp2.tile([P, ZW], F32, tag="z2")
                    for hh in range(ZW // 512):
                        nc.tensor.matmul(
                            z2[:, 512 * hh:512 * (hh + 1)], W2diag[blk][:],
                            a1[:, 512 * hh:512 * (hh + 1)],
                        )
                    if last:
                        nc.scalar.activation(
                            h_dst[:, ZW * t:ZW * (t + 1)], z2[:],
                            AF.Gelu, bias=b2rep[:, blk:blk + 1],
                            accum_out=pooled_parts[:, t:t + 1],
                        )
                    else:
                        nc.scalar.activation(
                            h_dst[:, ZW * t:ZW * (t + 1)], z2[:],
                            AF.Gelu, bias=b2rep[:, blk:blk + 1],
                        )
                        # next block's token-major transpose for slice t//2
                        # fires as soon as its two chunks are written
                        if t % 2 == 1:
                            hx_next.append(_new_hx(h_dst, t // 2))
                        if t == NZ // 2 - 1:
                            for s in range(NSL // 2):
                                _p1_slice(blk + 1, s, hx_next[s], G_next)
                if not last:
                    for s in range(NSL // 2, NSL):
                        _p1_slice(blk + 1, s, hx_next[s], G_next)
                    G_cur = G_next
                if DEBUG and _rep == 0:
                    nc.sync.dma_start(dbg_h[blk][:], h_dst[:, 0:ZW])

            # ===================== head =================================
            if _rep == 0:
                # head-only params: emitted late so their DMA triggers queue
                # behind the main-loop transposes on the sync queue
                Whrep = _load_rep(nc, cst, Wh_d[:], C, "wh")
                Wfrep = _load_rep(nc, cst, Wf_d[:], DOUT, "wf")
                bf_s = cst.tile([P, 1], F32)
                nc.vector.memset(bf_s[:], 0.0)
                for b in range(B_LOC):
                    nc.sync.dma_start(
                        bf_s[C * b:C * b + DOUT, :], bf_d[:].unsqueeze(-1))
            if DEBUG and _rep == 0:
                nc.sync.dma_start(dbg_pool[:], pooled_parts[:])
            psum_ = smal.tile([P, 1], F32, tag="poolsum")
            nc.vector.reduce_sum(psum_[:], pooled_parts[:],
                                 axis=mybir.AxisListType.X)
            y_ps = tps.tile([P, C], F32, tag="tiny")
            for b in range(B_LOC):
                sl = slice(C * b, C * (b + 1))
                nc.tensor.matmul(
                    y_ps[sl, 0:1], Whrep[sl, :], psum_[sl, :],
                    tile_position=(C * b, C * b),
                )
            ybn = smal.tile([P, 1], F32, tag="ybn")
            nc.vector.tensor_scalar(
                ybn[:], y_ps[:, 0:1], svecL[:], tvec[:], op0=MUL, op1=ADD,
            )
            yg = smal.tile([P, 1], F32, tag="yg")
            nc.scalar.activation(yg[:], ybn[:], AF.Gelu)
            o_ps = tps.tile([P, C], F32, tag="tiny")
            for b in range(B_LOC):
                nc.tensor.matmul(
                    o_ps[C * b:C * b + DOUT, 0:1],
                    Wfrep[C * b:C * (b + 1), :],
                    yg[C * b:C * (b + 1), :],
                    tile_position=(C * b, C * b),
                )
            ob = smal.tile([P, 1], F32, tag="ob")
            for b in range(B_LOC):
                sl = slice(C * b, C * b + DOUT)
                nc.vector.tensor_tensor(ob[sl, :], o_ps[sl, 0:1], bf_s[sl, :],
                                        op=ADD)
            for b in range(B_LOC):
                nc.sync.dma_start(
                    out_d[b, :], ob[C * b:C * b + DOUT, 0],
                )

    _fix_sync_waits(nc)
    return nc


_NC_CACHE = [None]


def kernel(**inputs) -> np.ndarray:
    arrs = {k: np.asarray(v, dtype=np.float32) for k, v in inputs.items()}
    x = arrs["x"]
    B = x.shape[0]
    n_cores = 8
    bl = B // n_cores

    if _NC_CACHE[0] is None:
        _NC_CACHE[0] = build_program()
    nc = _NC_CACHE[0]

    params = {k: arrs[k] for k in (
        "fcW1", "fcb1", "fcW2", "fcb2", "Wh", "bh",
        "bn_gamma", "bn_beta", "bn_mean", "bn_var", "Wf", "bf")}
    in_maps = [
        {"x": np.ascontiguousarray(x[bl * i: bl * (i + 1)]), **params}
        for i in range(n_cores)
    ]
    res = run_bass_kernel_spmd(nc, in_maps, list(range(n_cores))).results
    return np.concatenate([res[i]["out"] for i in range(n_cores)], axis=0)
